# revision 1
# baseline (speedup 1.0000x reference)
"""PiCANet-G attention module as a Trainium2 Bass/Tile kernel.

Pure data-parallel over batch: 64 samples -> 8 cores x 8 samples.

Per core, three phases (all SBUF-resident, bf16 matmuls, fp32 cell state):
  P1: vertical bi-LSTM over W (batch = 8*28 (b, h) rows, 28 steps, 2 dirs)
  P2: horizontal bi-LSTM over H (batch = 8*28 (b, w) rows)
  P3: fc -> softmax(100) -> per-sample einsum with the dilated 10x10 patch

Recurrence layout: gates G[1024, 224] with the gate dim on partitions
(8 m-tiles packed pairwise into 4 PSUM banks); hidden state h[256, 224]
is produced directly in the layout the next step's matmul consumes (rhs
with K on partitions) so there are no per-step transposes. Weights are
pre-transposed/permuted on the host (not part of device exec time).
"""

import numpy as np
import ml_dtypes
from contextlib import ExitStack

import concourse.bacc as bacc
import concourse.mybir as mybir
import concourse.tile as tile
from concourse.masks import make_identity
from concourse.bass_utils import run_bass_kernel_spmd

# problem shapes (hardcoded per contract)
B, C, H, W = 64, 512, 28, 28
HID = 256
N_CORES = 8
BL = B // N_CORES        # samples per core
NB = BL * H              # 224 rows per LSTM step
T = 28                   # steps per LSTM
PLOC = BL * H * W        # 6272 positions per core

BF16 = mybir.dt.bfloat16
F32 = mybir.dt.float32
AF = mybir.ActivationFunctionType

# torch gate order [i f g o] -> device order [i f o g] (sigmoids first)
_PERM = np.concatenate([np.arange(0, 512), np.arange(768, 1024), np.arange(512, 768)])
_GATE_FUNC = [AF.Sigmoid, AF.Sigmoid, AF.Sigmoid, AF.Tanh]

_LSTMS = ["vf", "vb", "hf", "hb"]


def _emit_lstm_step(nc, gpool, scr, wih_sb, whh_sb, bias_sb, src_rhs, dst_slab,
                    c_ap, dir_i, t, name, has_bias=True):
    """One LSTM step for one direction. src_rhs(kk, pos) -> [128, 224] AP."""
    pos = t if dir_i == 0 else T - 1 - t
    prev = pos - 1 if dir_i == 0 else pos + 1
    gates = []
    for gate in range(4):
        gt = gpool.tile([128, 512], F32, tag=f"g{gate}", name=f"g_{name}_{t}_{gate}")
        for half in range(2):
            m = gate * 2 + half
            out_ap = gt[:, half * 256: half * 256 + 224]
            for kk in range(4):
                nc.tensor.matmul(
                    out_ap,
                    lhsT=wih_sb[:, kk, m * 128:(m + 1) * 128],
                    rhs=src_rhs(kk, pos),
                    start=(half == 0 and kk == 0),
                    stop=(t == 0 and half == 1 and kk == 3),
                )
            if t > 0:
                for kk in range(2):
                    nc.tensor.matmul(
                        out_ap,
                        lhsT=whh_sb[:, kk, m * 128:(m + 1) * 128],
                        rhs=dst_slab[:, dir_i * 2 + kk, prev * 224:(prev + 1) * 224],
                        start=False,
                        stop=(half == 1 and kk == 1),
                    )
        gv = gt.rearrange("p (two x) -> p two x", two=2)[:, :, 0:224]
        if gate == 3:
            # tanh(g) to SBUF so the i*g product has only one PSUM operand
            tg = scr.tile([128, 2, 224], F32, tag="tg", bufs=3,
                          name=f"tg_{name}_{t}")
            if has_bias:
                for half in range(2):
                    m = gate * 2 + half
                    nc.scalar.activation(tg[:, half, :], gv[:, half, :],
                                         _GATE_FUNC[gate],
                                         bias=bias_sb[:, m:m + 1])
            else:
                nc.scalar.activation(tg, gv, _GATE_FUNC[gate])
            gates.append(tg)
        else:
            if has_bias:
                for half in range(2):
                    m = gate * 2 + half
                    # fused bias + nonlinearity, in place in PSUM
                    nc.scalar.activation(gv[:, half, :], gv[:, half, :],
                                         _GATE_FUNC[gate],
                                         bias=bias_sb[:, m:m + 1])
            else:
                # biases all zero: one activation over both halves (gap skipped)
                nc.scalar.activation(gv, gv, _GATE_FUNC[gate])
            gates.append(gv)
    g_i, g_f, g_o, g_g = gates

    if t == 0:
        nc.vector.tensor_mul(c_ap, g_i, g_g)
    else:
        t1 = scr.tile([128, 2, 224], F32, tag="t1", bufs=3, name=f"t1_{name}_{t}")
        nc.vector.tensor_mul(t1, g_i, g_g)
        nc.vector.tensor_mul(c_ap, g_f, c_ap)
        nc.vector.tensor_add(c_ap, c_ap, t1)
    th = scr.tile([128, 2, 224], F32, tag="th", bufs=3, name=f"th_{name}_{t}")
    nc.scalar.activation(th, c_ap, AF.Tanh)
    # h -> bf16 slab, both hidden halves in one strided write
    h_ap = dst_slab[:, dir_i * 2:dir_i * 2 + 2, pos * 224:(pos + 1) * 224]
    nc.vector.tensor_mul(h_ap, g_o, th)


def _build(reps=1, debug=False, has_bias=True):
    nc = bacc.Bacc(None, target_bir_lowering=False)

    xT_d = nc.dram_tensor("xT", [C, PLOC], BF16, kind="ExternalInput")
    w_d = {}
    for L in _LSTMS:
        w_d[L + "_wih"] = nc.dram_tensor(L + "_wih", [512, 1024], BF16, kind="ExternalInput")
        w_d[L + "_whh"] = nc.dram_tensor(L + "_whh", [256, 1024], BF16, kind="ExternalInput")
        w_d[L + "_bias"] = nc.dram_tensor(L + "_bias", [128, 8], F32, kind="ExternalInput")
    fcw_d = nc.dram_tensor("fcw", [512, 100], BF16, kind="ExternalInput")
    fcb_d = nc.dram_tensor("fcb", [1, 100], BF16, kind="ExternalInput")
    patchT_d = nc.dram_tensor("patchT", [BL, 100, 512], BF16, kind="ExternalInput")
    out_d = nc.dram_tensor("out", [BL, C, H * W], F32, kind="ExternalOutput")
    if debug:
        dbg_hv = nc.dram_tensor("dbg_hv", [128, 4, PLOC], BF16, kind="ExternalOutput")
        dbg_hh = nc.dram_tensor("dbg_hh", [128, 4, PLOC], BF16, kind="ExternalOutput")
        dbg_kt = nc.dram_tensor("dbg_kt", [100, PLOC], BF16, kind="ExternalOutput")

    with tile.TileContext(nc) as tc, ExitStack() as ctx:
        wpool = ctx.enter_context(tc.tile_pool(name="wpool", bufs=1))
        bigA = ctx.enter_context(tc.tile_pool(name="bigA", bufs=1))
        bigB = ctx.enter_context(tc.tile_pool(name="bigB", bufs=1))
        state = ctx.enter_context(tc.tile_pool(name="state", bufs=1))
        scr = ctx.enter_context(tc.tile_pool(name="scr", bufs=3))

        # --- load weights; both stage-1 dirs first (step 0 needs them) ---
        wih_sb, whh_sb, bias_sb = {}, {}, {}
        for L in _LSTMS:
            wih_sb[L] = wpool.tile([128, 4, 1024], BF16, name=f"wih_{L}")
            whh_sb[L] = wpool.tile([128, 2, 1024], BF16, name=f"whh_{L}")
            bias_sb[L] = wpool.tile([128, 8], F32, name=f"bias_{L}")
        for L in ["vf", "vb"]:
            nc.sync.dma_start(out=wih_sb[L],
                              in_=w_d[L + "_wih"].rearrange("(kt p) m -> p kt m", kt=4))
        for L in ["vf", "vb"]:
            nc.scalar.dma_start(out=whh_sb[L],
                                in_=w_d[L + "_whh"].rearrange("(kt p) m -> p kt m", kt=2))
            if has_bias:
                nc.scalar.dma_start(out=bias_sb[L], in_=w_d[L + "_bias"][:, :])
        for L in ["hf", "hb"]:
            nc.sync.dma_start(out=wih_sb[L],
                              in_=w_d[L + "_wih"].rearrange("(kt p) m -> p kt m", kt=4))
            nc.sync.dma_start(out=whh_sb[L],
                              in_=w_d[L + "_whh"].rearrange("(kt p) m -> p kt m", kt=2))
            if has_bias:
                nc.sync.dma_start(out=bias_sb[L], in_=w_d[L + "_bias"][:, :])
        fcw_sb = wpool.tile([128, 4, 100], BF16, name="fcw_sb")
        nc.sync.dma_start(out=fcw_sb, in_=fcd_rearr(fcw_d))
        if has_bias:
            fcb_sb = wpool.tile([1, 100], BF16, name="fcb_sb")
            nc.sync.dma_start(out=fcb_sb, in_=fcb_d[:, :])
            ones112 = wpool.tile([1, 112], BF16, name="ones112")
            nc.vector.memset(ones112, 1.0)
        else:
            fcb_sb = ones112 = None
        patchT_sb = wpool.tile([100, BL, 512], BF16, name="patchT_sb")
        nc.sync.dma_start(out=patchT_sb, in_=patchT_d.rearrange("b k c -> k b c"))
        ident = wpool.tile([112, 112], F32, name="ident")
        make_identity(nc, ident)

        for rep in range(reps):
            sfx = f"r{rep}"
            # --- P1: vertical bi-LSTM ---
            xT = bigA.tile([128, 4, PLOC], BF16, tag="bigA", name=f"xT_{sfx}")
            xsrc = xT_d.rearrange("(kt p) f -> p kt f", kt=4)
            # stream in the order both directions consume: edges first
            wblocks = [(0, 3), (25, 28), (3, 8), (20, 25), (8, 14), (14, 20)]
            for lo, hi in wblocks:
                for kk in range(4):
                    nc.scalar.dma_start(out=xT[:, kk, lo * 224:hi * 224],
                                        in_=xsrc[:, kk, lo * 224:hi * 224])
            Hv = bigB.tile([128, 4, PLOC], BF16, tag="bigB", name=f"Hv_{sfx}")

            def rhs1(kk, pos, _xT=xT):
                # xT free layout is (w, b, h): one contiguous slice per step
                return _xT[:, kk, pos * 224:(pos + 1) * 224]

            with tc.tile_pool(name="gates1", bufs=2, space="PSUM") as gpool:
                cs = [state.tile([128, 2, 224], F32, tag=f"c1_{d}",
                                 name=f"c1_{d}_{sfx}") for d in range(2)]
                for t in range(T):
                    for d, L in enumerate(["vf", "vb"]):
                        _emit_lstm_step(nc, gpool, scr, wih_sb[L], whh_sb[L],
                                        bias_sb[L], rhs1, Hv, cs[d], d, t,
                                        f"1{L}{sfx}", has_bias=has_bias)

            # --- P2: horizontal bi-LSTM ---
            Hh = bigA.tile([128, 4, PLOC], BF16, tag="bigA", name=f"Hh_{sfx}")

            def rhs2(kk, pos, _Hv=Hv):
                a = _Hv[:, kk, :].rearrange("p (w b h) -> p b w h", w=W, b=BL)
                return a[:, :, :, pos]

            with tc.tile_pool(name="gates2", bufs=2, space="PSUM") as gpool:
                cs = [state.tile([128, 2, 224], F32, tag=f"c2_{d}",
                                 name=f"c2_{d}_{sfx}") for d in range(2)]
                for t in range(T):
                    for d, L in enumerate(["hf", "hb"]):
                        _emit_lstm_step(nc, gpool, scr, wih_sb[L], whh_sb[L],
                                        bias_sb[L], rhs2, Hh, cs[d], d, t,
                                        f"2{L}{sfx}", has_bias=has_bias)

            # --- P3: fc + softmax + transpose + einsum ---
            KT = bigB.tile([100, PLOC], BF16, tag="bigB", name=f"KT_{sfx}")
            with tc.tile_pool(name="p3ps", bufs=2, space="PSUM") as pps:
                ci = 0
                for half in range(2):
                    # fc + softmax + transpose for samples b in 4*half..4*half+3
                    for hr in range(H):
                        off = hr * 224 + half * 112
                        Lp = pps.tile([112, 100], F32, tag="L", name=f"L_{hr}_{half}_{sfx}")
                        for kk in range(4):
                            lhsT = Hh[:, kk, off:off + 112]
                            nc.tensor.matmul(Lp, lhsT=lhsT, rhs=fcw_sb[:, kk, :],
                                             start=(kk == 0),
                                             stop=(not has_bias and kk == 3))
                        if has_bias:
                            nc.tensor.matmul(Lp, lhsT=ones112, rhs=fcb_sb,
                                             start=False, stop=True)
                        E = scr.tile([112, 100], F32, tag="E", bufs=3,
                                     name=f"E_{hr}_{half}_{sfx}")
                        Zs = scr.tile([112, 1], F32, tag="Z", bufs=3,
                                      name=f"Z_{hr}_{half}_{sfx}")
                        nc.scalar.activation(E, Lp, AF.Exp, accum_out=Zs)
                        rz = scr.tile([112, 1], F32, tag="rz", bufs=3,
                                      name=f"rz_{hr}_{half}_{sfx}")
                        nc.vector.reciprocal(rz, Zs)
                        Ka = scr.tile([112, 100], F32, tag="Ka", bufs=3,
                                      name=f"Ka_{hr}_{half}_{sfx}")
                        nc.vector.tensor_scalar_mul(Ka, E, rz)
                        KTp = pps.tile([100, 112], F32, tag="KTp",
                                       name=f"KTp_{hr}_{half}_{sfx}")
                        nc.tensor.transpose(KTp, Ka, ident)
                        # KT columns p = b*784 + hr*28 + w for these positions
                        dst = KT.rearrange("k (b hw) -> k b hw", b=BL)[
                            :, half * 4:(half + 1) * 4, hr * 28:(hr + 1) * 28]
                        if ci % 2 == 0:
                            nc.vector.tensor_copy(dst, KTp)
                        else:
                            nc.scalar.copy(dst, KTp)
                        ci += 1
                    # einsum for this half's samples (overlaps the other half's fc)
                    for b_i in range(half * 4, (half + 1) * 4):
                        for ct in range(4):
                            lhsT = patchT_sb[:, b_i, ct * 128:(ct + 1) * 128]
                            # [128, 1024] = 2 PSUM banks; each matmul output
                            # must stay inside one bank, so halves go at 0/512
                            Op = pps.tile([128, 2, 512], F32, tag="O", bufs=2,
                                          name=f"O_{b_i}_{ct}_{sfx}")
                            for j2 in range(2):
                                nc.tensor.matmul(
                                    Op[:, j2, 0:392], lhsT=lhsT,
                                    rhs=KT[:, b_i * 784 + j2 * 392:
                                           b_i * 784 + (j2 + 1) * 392],
                                    start=True, stop=True)
                            ob = scr.tile([128, 2, 392], F32, tag="ob", bufs=3,
                                          name=f"ob_{b_i}_{ct}_{sfx}")
                            if ct % 2 == 0:
                                nc.vector.tensor_copy(ob, Op[:, :, 0:392])
                            else:
                                nc.scalar.copy(ob, Op[:, :, 0:392])
                            eng = nc.sync if ct % 2 == 0 else nc.scalar
                            eng.dma_start(
                                out=out_d[b_i, ct * 128:(ct + 1) * 128, :],
                                in_=ob)
            if debug and rep == reps - 1:
                nc.sync.dma_start(out=dbg_hv[:, :, :], in_=Hv)
                nc.sync.dma_start(out=dbg_hh[:, :, :], in_=Hh)
                nc.sync.dma_start(out=dbg_kt[:, :], in_=KT)

    nc.compile()
    return nc


def fcd_rearr(fcw_d):
    return fcw_d.rearrange("(kt p) n -> p kt n", kt=4)


_NC_CACHE = {}


def _get_nc(reps=1, debug=False, has_bias=True):
    key = (reps, debug, has_bias)
    if key not in _NC_CACHE:
        _NC_CACHE[key] = _build(reps=reps, debug=debug, has_bias=has_bias)
    return _NC_CACHE[key]


def _prep_core_inputs(x, weights_np):
    """Host-side marshalling for one core. x: [BL, C, H, W] f32."""
    bf = ml_dtypes.bfloat16
    m = {}
    m["xT"] = np.ascontiguousarray(
        x.transpose(1, 3, 0, 2).reshape(C, PLOC)).astype(bf)
    m["patchT"] = np.ascontiguousarray(
        x[:, :, ::3, ::3].reshape(BL, C, 100).transpose(0, 2, 1)).astype(bf)
    m.update(weights_np)
    return m


def _prep_weights(inputs):
    bf = ml_dtypes.bfloat16
    w = {}
    for L in _LSTMS:
        wih = np.asarray(inputs[L + "_Wih"], np.float32)
        whh = np.asarray(inputs[L + "_Whh"], np.float32)
        bih = np.asarray(inputs[L + "_bih"], np.float32)
        bhh = np.asarray(inputs[L + "_bhh"], np.float32)
        w[L + "_wih"] = np.ascontiguousarray(wih[_PERM].T).astype(bf)
        w[L + "_whh"] = np.ascontiguousarray(whh[_PERM].T).astype(bf)
        w[L + "_bias"] = np.ascontiguousarray(
            (bih + bhh)[_PERM].reshape(8, 128).T).astype(np.float32)
    w["fcw"] = np.asarray(inputs["fc_W"], np.float32).astype(bf)
    w["fcb"] = np.asarray(inputs["fc_b"], np.float32).reshape(1, 100).astype(bf)
    return w


def run_cores(inputs, reps=1, debug=False):
    x = np.asarray(inputs["x"], np.float32)
    wnp = _prep_weights(inputs)
    has_bias = any(np.any(wnp[L + "_bias"]) for L in _LSTMS)
    nc = _get_nc(reps=reps, debug=debug, has_bias=has_bias)
    in_maps = [
        _prep_core_inputs(x[ci * BL:(ci + 1) * BL], wnp) for ci in range(N_CORES)
    ]
    res = run_bass_kernel_spmd(nc, in_maps, list(range(N_CORES)))
    return res


def kernel(**inputs) -> np.ndarray:
    res = run_cores(inputs)
    out = np.concatenate(
        [res.results[ci]["out"].reshape(BL, C, H, W) for ci in range(N_CORES)],
        axis=0)
    return out.astype(np.float32)



# revision 13
# speedup vs baseline: 2.4131x; 2.4131x over previous
"""PiCANet-G attention module as a Trainium2 Bass/Tile kernel (fp8 DoubleRow).

Pure data-parallel over batch: 64 samples -> 8 cores x 8 samples.

Per core, three phases:
  P1: vertical bi-LSTM over W (batch = 8*28 (b, h) rows, 28 steps, 2 dirs)
  P2: horizontal bi-LSTM over H (batch = 8*28 (b, w) rows)
  P3: fc -> softmax(100) -> per-sample einsum with the dilated 10x10 patch

All LSTM/fc matmuls run in fp8 e4m3 with perf_mode=DoubleRow: the K=256
contraction is packed as pairs along the free dim, so each gate needs only
3 matmul instructions (2 ih + 1 hh) per 128-wide m-tile.

Gate nonlinearities are all Tanh (one batched activation per step/dir over
the 4 gate PSUM banks): sigma(x) = 0.5 + 0.5*tanh(x/2) is realized by
pre-scaling sigma-gate weight rows x16 and g-gate rows x32 on the host and
applying activation scale 1/32.  The cell update then uses fused
scalar_tensor_tensor ops with the same op count as the naive form:
    m = (tau_i + 1) * tau_g          C_0 = m            (C := 2c)
    n = (tau_f + 1) * C_prev         C   = 0.5n + m
    h'' = (tau_o + 1) * tanh(C/2)    (h'' = 2h; consumers' weights x0.5)
h'' is stored fp8 twice in stage 1: contiguous (for the stage's own hh
matmul) and strided into the next stage's DoubleRow pair layout.
"""

import numpy as np
import ml_dtypes
from contextlib import ExitStack

import concourse.bacc as bacc
import concourse.mybir as mybir
import concourse.tile as tile
from concourse.masks import make_identity
from concourse.bass_utils import run_bass_kernel_spmd

# problem shapes (hardcoded per contract)
B, C, H, W = 64, 512, 28, 28
HID = 256
N_CORES = 8
BL = B // N_CORES        # samples per core
NB = BL * H              # 224 rows per LSTM step
T = 28                   # steps per LSTM
PLOC = BL * H * W        # 6272 positions per core

BF16 = mybir.dt.bfloat16
F32 = mybir.dt.float32
F8 = mybir.dt.float8e4
AF = mybir.ActivationFunctionType
ALU = mybir.AluOpType
PM = mybir.MatmulPerfMode
F8NP = ml_dtypes.float8_e4m3
BFNP = ml_dtypes.bfloat16

# torch gate order [i f g o] -> device order [i f o g]
_PERM = np.concatenate([np.arange(0, 512), np.arange(768, 1024), np.arange(512, 768)])
_LSTMS = ["vf", "vb", "hf", "hb"]


def _emit_ih(nc, gt, wih_sb, src, pos, t):
    """Input projection: 2 DoubleRow matmuls per m-tile (K=512).

    PSUM zeroing (start=True) is bank-granular: exactly one start per gate
    bank — on the even m-tile's first matmul, emitted before the odd m-tile
    touches the bank.  The odd tile's first write lands on pending-zero
    bytes, which is the fresh-write behavior we want.
    """
    for m in range(8):
        out = gt[:, m, 0:224]
        for kt in range(2):
            nc.tensor.matmul(
                out,
                lhsT=wih_sb[:, kt, :, m * 128:(m + 1) * 128],
                rhs=src(kt, pos),
                start=(m % 2 == 0 and kt == 0),
                stop=(t == 0 and m % 2 == 1 and kt == 1),
                skip_group_check=True,
                perf_mode=PM.DoubleRow,
            )


def _emit_hh(nc, gt, whh_sb, hslab, d, prev):
    """Recurrent projection: 1 DoubleRow matmul per m-tile (K=256)."""
    rhs = hslab[:, 2 * d:2 * d + 2, prev * 224:(prev + 1) * 224]
    for m in range(8):
        nc.tensor.matmul(
            gt[:, m, 0:224],
            lhsT=whh_sb[:, :, m * 128:(m + 1) * 128],
            rhs=rhs,
            start=False,
            stop=(m % 2 == 1),
            skip_group_check=True,
            perf_mode=PM.DoubleRow,
        )


def _emit_cell(nc, scr, gt, cs, bias_sb, t, d, name, h1, h2halves, has_bias):
    """Gate activations + cell update + h'' writes for one (step, dir).

    h1: contiguous [128, 2, 224] slab view for this stage's own hh matmul.
    h2halves: optional pair of [128, 8, 28] strided views (next stage's
    DoubleRow pair layout), one per hidden half.
    """
    tau = scr.tile([128, 8, 224], BF16, tag=f"tau{d}", bufs=3,
                   name=f"tau_{name}")
    if has_bias:
        for m in range(8):
            nc.scalar.activation(tau[:, m], gt[:, m, 0:224],
                                 AF.Tanh, scale=1.0 / 32.0,
                                 bias=bias_sb[:, m:m + 1])
    else:
        nc.scalar.activation(tau, gt[:, :, 0:224], AF.Tanh, scale=1.0 / 32.0)

    if t == 0:
        # C = m = (tau_i + 1) * tau_g
        nc.vector.scalar_tensor_tensor(cs, tau[:, 0:2], 1.0, tau[:, 6:8],
                                       ALU.add, ALU.mult)
    else:
        mt = scr.tile([128, 2, 224], BF16, tag=f"m{d}", bufs=3,
                      name=f"m_{name}")
        nc.vector.scalar_tensor_tensor(mt, tau[:, 0:2], 1.0, tau[:, 6:8],
                                       ALU.add, ALU.mult)
        nt = scr.tile([128, 2, 224], F32, tag=f"n{d}", bufs=3,
                      name=f"n_{name}")
        # n = (tau_f + 1) * C_prev ; C = 0.5 n + m
        nc.vector.scalar_tensor_tensor(nt, tau[:, 2:4], 1.0, cs, ALU.add, ALU.mult)
        nc.vector.scalar_tensor_tensor(cs, nt, 0.5, mt, ALU.mult, ALU.add)
    tc = scr.tile([128, 2, 224], BF16, tag=f"tc{d}", bufs=3, name=f"tc_{name}")
    nc.scalar.activation(tc, cs, AF.Tanh, scale=0.5)
    # h'' = (tau_o + 1) * tanh(c)
    nc.vector.scalar_tensor_tensor(h1, tau[:, 4:6], 1.0, tc,
                                   ALU.add, ALU.mult)
    if h2halves:
        # gpsimd re-layouts h'' from the contiguous slab into the next
        # stage's DoubleRow pair layout (neuronxcc: no stt on Pool)
        for dst, src in h2halves:
            nc.gpsimd.tensor_copy(dst, src)


def _build(reps=1, debug=False, has_bias=False):
    nc = bacc.Bacc(None, target_bir_lowering=False)

    xT_d = nc.dram_tensor("xT", [128, 2, 2, PLOC], F8, kind="ExternalInput")
    w_d = {}
    for L in _LSTMS:
        w_d[L + "_wih"] = nc.dram_tensor(L + "_wih", [128, 2, 2, 1024], F8,
                                         kind="ExternalInput")
        w_d[L + "_whh"] = nc.dram_tensor(L + "_whh", [128, 2, 1024], F8,
                                         kind="ExternalInput")
        if has_bias:
            w_d[L + "_bias"] = nc.dram_tensor(L + "_bias", [128, 8], F32,
                                              kind="ExternalInput")
    fcw_d = nc.dram_tensor("fcw", [128, 2, 2, 112], F8, kind="ExternalInput")
    patchT_d = nc.dram_tensor("patchT", [BL, 100, 512], BF16,
                              kind="ExternalInput")
    out_d = nc.dram_tensor("out", [BL, C, H * W], BF16, kind="ExternalOutput")
    if debug:
        dbg_hv1 = nc.dram_tensor("dbg_hv1", [128, 4, PLOC], F8, kind="ExternalOutput")
        dbg_hv2 = nc.dram_tensor("dbg_hv2", [128, 4, PLOC], F8, kind="ExternalOutput")
        dbg_hh = nc.dram_tensor("dbg_hh", [128, 4, PLOC], F8, kind="ExternalOutput")
        dbg_kt = nc.dram_tensor("dbg_kt", [100, PLOC], BF16, kind="ExternalOutput")

    with tile.TileContext(nc) as tc, ExitStack() as ctx:
        wpool = ctx.enter_context(tc.tile_pool(name="wpool", bufs=1))
        slabA = ctx.enter_context(tc.tile_pool(name="slabA", bufs=1))
        slabB = ctx.enter_context(tc.tile_pool(name="slabB", bufs=1))
        slabC = ctx.enter_context(tc.tile_pool(name="slabC", bufs=1))
        state = ctx.enter_context(tc.tile_pool(name="state", bufs=1))
        scr = ctx.enter_context(tc.tile_pool(name="scr", bufs=3))

        # --- weights ---
        wih_sb, whh_sb, bias_sb = {}, {}, {}
        for L in _LSTMS:
            wih_sb[L] = wpool.tile([128, 2, 2, 1024], F8, name=f"wih_{L}")
            whh_sb[L] = wpool.tile([128, 2, 1024], F8, name=f"whh_{L}")
            bias_sb[L] = wpool.tile([128, 8], F32, name=f"bias_{L}") if has_bias else None
        for L in ["vf", "vb"]:
            nc.sync.dma_start(out=wih_sb[L], in_=w_d[L + "_wih"][:, :, :, :])
            nc.sync.dma_start(out=whh_sb[L], in_=w_d[L + "_whh"][:, :, :])
            if has_bias:
                nc.sync.dma_start(out=bias_sb[L], in_=w_d[L + "_bias"][:, :])
        for L in ["hf", "hb"]:
            nc.scalar.dma_start(out=wih_sb[L], in_=w_d[L + "_wih"][:, :, :, :])
            nc.scalar.dma_start(out=whh_sb[L], in_=w_d[L + "_whh"][:, :, :])
            if has_bias:
                nc.scalar.dma_start(out=bias_sb[L], in_=w_d[L + "_bias"][:, :])
        fcw_sb = wpool.tile([128, 2, 2, 112], F8, name="fcw_sb")
        nc.scalar.dma_start(out=fcw_sb, in_=fcw_d[:, :, :, :])
        patchT_sb = wpool.tile([100, BL, 512], BF16, name="patchT_sb")
        nc.scalar.dma_start(out=patchT_sb, in_=patchT_d.rearrange("b k c -> k b c"))
        ident = wpool.tile([112, 112], F32, name="ident")
        make_identity(nc, ident)

        for rep in range(reps):
            sfx = f"r{rep}"
            # --- xT stream-in, edges first (vb consumes w=27 at t=0) ---
            xT = slabA.tile([128, 2, 2, PLOC], F8, tag="slabA", name=f"xT_{sfx}")
            wblocks = [(0, 3), (25, 28), (3, 8), (20, 25), (8, 14), (14, 20)]
            for bi, (lo, hi) in enumerate(wblocks):
                eng = nc.gpsimd if bi % 2 == 0 else nc.sync
                eng.dma_start(out=xT[:, :, :, lo * 224:hi * 224],
                              in_=xT_d[:, :, :, lo * 224:hi * 224])

            Hv1 = slabB.tile([128, 4, PLOC], F8, tag="slabB", name=f"Hv1_{sfx}")
            Hv2 = slabC.tile([128, 4, PLOC], F8, tag="slabC", name=f"Hv2_{sfx}")
            # Hv2 free layout is (h, b, w); strided write view iterates (b, h)
            Hv2v = Hv2.rearrange("p kk (h b w) -> p kk b h w", h=28, b=BL)

            def src1(kt, pos, _xT=xT):
                return _xT[:, kt, :, pos * 224:(pos + 1) * 224]

            # --- P1: vertical bi-LSTM ---
            with tc.tile_pool(name="gates1", bufs=1, space="PSUM") as gpool:
                cs = [state.tile([128, 2, 224], F32, tag=f"c1_{d}",
                                 name=f"c1_{d}_{sfx}") for d in range(2)]
                for t in range(T):
                    gts = []
                    for d in range(2):
                        pos = t if d == 0 else T - 1 - t
                        gt = gpool.tile([128, 8, 256], F32, tag=f"g{d}",
                                        name=f"g1_{d}_{t}_{sfx}")
                        gts.append(gt)
                        _emit_ih(nc, gt, wih_sb["vf" if d == 0 else "vb"],
                                 src1, pos, t)
                    for d in range(2):
                        pos = t if d == 0 else T - 1 - t
                        prev = pos - 1 if d == 0 else pos + 1
                        if t > 0:
                            _emit_hh(nc, gts[d], whh_sb["vf" if d == 0 else "vb"],
                                     Hv1, d, prev)
                    for d in range(2):
                        pos = t if d == 0 else T - 1 - t
                        L = "vf" if d == 0 else "vb"
                        h1 = Hv1[:, 2 * d:2 * d + 2, pos * 224:(pos + 1) * 224]
                        h2h = [(Hv2v[:, 2 * d + i, :, :, pos],
                                Hv1[:, 2 * d + i, pos * 224:(pos + 1) * 224])
                               for i in range(2)]
                        _emit_cell(nc, scr, gts[d], cs[d], bias_sb[L], t, d,
                                   f"1{L}{t}{sfx}", h1, h2h, has_bias)

            # --- P2: horizontal bi-LSTM ---
            Hh = slabA.tile([128, 4, PLOC], F8, tag="slabA", name=f"Hh_{sfx}")

            def src2(kt, pos, _Hv2=Hv2):
                return _Hv2[:, 2 * kt:2 * kt + 2, pos * 224:(pos + 1) * 224]

            with tc.tile_pool(name="gates2", bufs=1, space="PSUM") as gpool:
                cs = [state.tile([128, 2, 224], F32, tag=f"c2_{d}",
                                 name=f"c2_{d}_{sfx}") for d in range(2)]
                for t in range(T):
                    gts = []
                    for d in range(2):
                        pos = t if d == 0 else T - 1 - t
                        gt = gpool.tile([128, 8, 256], F32, tag=f"g{d}",
                                        name=f"g2_{d}_{t}_{sfx}")
                        gts.append(gt)
                        _emit_ih(nc, gt, wih_sb["hf" if d == 0 else "hb"],
                                 src2, pos, t)
                    for d in range(2):
                        pos = t if d == 0 else T - 1 - t
                        prev = pos - 1 if d == 0 else pos + 1
                        if t > 0:
                            _emit_hh(nc, gts[d], whh_sb["hf" if d == 0 else "hb"],
                                     Hh, d, prev)
                    for d in range(2):
                        pos = t if d == 0 else T - 1 - t
                        L = "hf" if d == 0 else "hb"
                        h1 = Hh[:, 2 * d:2 * d + 2, pos * 224:(pos + 1) * 224]
                        _emit_cell(nc, scr, gts[d], cs[d], bias_sb[L], t, d,
                                   f"2{L}{t}{sfx}", h1, None, has_bias)

            # --- P3: fc + softmax + transpose + einsum ---
            KT = slabC.tile([100, PLOC], BF16, tag="slabC", name=f"KT_{sfx}")
            KTv = KT.rearrange("k (b hw) -> k b hw", b=BL)
            with tc.tile_pool(name="p3ps", bufs=2, space="PSUM") as pps:
                ci = 0
                for half in range(2):
                    for s in range(H):
                        off = s * 224 + half * 112
                        Lp = pps.tile([112, 112], F32, tag="L",
                                      name=f"L_{s}_{half}_{sfx}")
                        for j in range(2):
                            nc.tensor.matmul(Lp, lhsT=Hh[:, 2 * j:2 * j + 2,
                                                         off:off + 112],
                                             rhs=fcw_sb[:, j],
                                             start=(j == 0), stop=(j == 1),
                                             perf_mode=PM.DoubleRow)
                        E = scr.tile([112, 100], F32, tag="E", bufs=3,
                                     name=f"E_{s}_{half}_{sfx}")
                        Zs = scr.tile([112, 1], F32, tag="Z", bufs=3,
                                      name=f"Z_{s}_{half}_{sfx}")
                        nc.scalar.activation(E, Lp[:, 0:100], AF.Exp,
                                             scale=1.0 / 16.0, accum_out=Zs)
                        rz = scr.tile([112, 1], F32, tag="rz", bufs=3,
                                      name=f"rz_{s}_{half}_{sfx}")
                        nc.vector.reciprocal(rz, Zs)
                        Ka = scr.tile([112, 100], F32, tag="Ka", bufs=3,
                                      name=f"Ka_{s}_{half}_{sfx}")
                        nc.vector.tensor_scalar_mul(Ka, E, rz)
                        KTp = pps.tile([100, 112], F32, tag="KTp",
                                       name=f"KTp_{s}_{half}_{sfx}")
                        nc.tensor.transpose(KTp, Ka, ident)
                        dst = KTv[:, half * 4:(half + 1) * 4,
                                  s * 28:(s + 1) * 28]
                        if ci % 2 == 0:
                            nc.vector.tensor_copy(dst, KTp[:, 0:112])
                        else:
                            nc.scalar.copy(dst, KTp[:, 0:112])
                        ci += 1
                    # einsum for this half's samples (overlaps other half's fc)
                    for b_i in range(half * 4, (half + 1) * 4):
                        for ct in range(4):
                            lhsT = patchT_sb[:, b_i, ct * 128:(ct + 1) * 128]
                            Op = pps.tile([128, 2, 512], F32, tag="O", bufs=2,
                                          name=f"O_{b_i}_{ct}_{sfx}")
                            for j2 in range(2):
                                nc.tensor.matmul(
                                    Op[:, j2, 0:392], lhsT=lhsT,
                                    rhs=KT[:, b_i * 784 + j2 * 392:
                                           b_i * 784 + (j2 + 1) * 392],
                                    start=True, stop=True)
                            ob = scr.tile([128, 2, 392], BF16, tag="ob", bufs=3,
                                          name=f"ob_{b_i}_{ct}_{sfx}")
                            if ct % 2 == 0:
                                nc.vector.tensor_copy(ob, Op[:, :, 0:392])
                            else:
                                nc.scalar.copy(ob, Op[:, :, 0:392])
                            eng = nc.sync if ct % 2 == 0 else nc.scalar
                            eng.dma_start(
                                out=out_d[b_i, ct * 128:(ct + 1) * 128, :],
                                in_=ob)
            if debug and rep == reps - 1:
                nc.sync.dma_start(out=dbg_hv1[:, :, :], in_=Hv1)
                nc.sync.dma_start(out=dbg_hv2[:, :, :], in_=Hv2)
                nc.sync.dma_start(out=dbg_hh[:, :, :], in_=Hh)
                nc.sync.dma_start(out=dbg_kt[:, :], in_=KT)

    nc.compile()
    return nc


_NC_CACHE = {}


def _get_nc(reps=1, debug=False, has_bias=False):
    key = (reps, debug, has_bias)
    if key not in _NC_CACHE:
        _NC_CACHE[key] = _build(reps=reps, debug=debug, has_bias=has_bias)
    return _NC_CACHE[key]


# row scale: sigma-gate rows (i,f,o) x16, g rows x32 (tanh-trick encoding)
_ROW_SCALE = np.concatenate([np.full(768, 16.0), np.full(256, 32.0)]).astype(np.float32)
# bias enters after the activation scale: b/2 for sigma rows, b for g rows
_BIAS_SCALE = np.concatenate([np.full(768, 0.5), np.full(256, 1.0)]).astype(np.float32)


def _prep_weights(inputs):
    w = {}
    for L in _LSTMS:
        wih = np.asarray(inputs[L + "_Wih"], np.float32)[_PERM]
        whh = np.asarray(inputs[L + "_Whh"], np.float32)[_PERM]
        in_half = 0.5 if L in ("hf", "hb") else 1.0  # stage-2 input is h''=2h
        wd = (wih * _ROW_SCALE[:, None] * in_half).T          # [din, 1024]
        din = wd.shape[0]
        wd = wd.reshape(din // 256, 2, 128, 1024).transpose(2, 0, 1, 3)
        w[L + "_wih"] = np.ascontiguousarray(wd).astype(F8NP)
        hd = (whh * _ROW_SCALE[:, None] * 0.5).T              # [256, 1024]
        hd = hd.reshape(2, 128, 1024).transpose(1, 0, 2)
        w[L + "_whh"] = np.ascontiguousarray(hd).astype(F8NP)
        bias = (np.asarray(inputs[L + "_bih"], np.float32)
                + np.asarray(inputs[L + "_bhh"], np.float32))[_PERM]
        w[L + "_bias"] = np.ascontiguousarray(
            (bias * _BIAS_SCALE).reshape(8, 128).T).astype(np.float32)
    fcw = np.asarray(inputs["fc_W"], np.float32) * 8.0        # [512, 100]
    fcw = np.pad(fcw, ((0, 0), (0, 12)))
    fcw = fcw.reshape(2, 2, 128, 112).transpose(2, 0, 1, 3)
    w["fcw"] = np.ascontiguousarray(fcw).astype(F8NP)
    return w


def _prep_core_inputs(x, weights_np, has_bias):
    """Host-side marshalling for one core. x: [BL, C, H, W] f32."""
    m = {}
    xt = x.transpose(1, 3, 0, 2).reshape(2, 2, 128, PLOC).transpose(2, 0, 1, 3)
    m["xT"] = np.ascontiguousarray(xt).astype(F8NP)
    m["patchT"] = np.ascontiguousarray(
        x[:, :, ::3, ::3].reshape(BL, C, 100).transpose(0, 2, 1)).astype(BFNP)
    for k, v in weights_np.items():
        if k.endswith("_bias") and not has_bias:
            continue
        m[k] = v
    return m


def run_cores(inputs, reps=1, debug=False):
    x = np.asarray(inputs["x"], np.float32)
    wnp = _prep_weights(inputs)
    has_bias = any(np.any(wnp[L + "_bias"]) for L in _LSTMS)
    nc = _get_nc(reps=reps, debug=debug, has_bias=has_bias)
    in_maps = [
        _prep_core_inputs(x[ci * BL:(ci + 1) * BL], wnp, has_bias)
        for ci in range(N_CORES)
    ]
    res = run_bass_kernel_spmd(nc, in_maps, list(range(N_CORES)))
    return res


def kernel(**inputs) -> np.ndarray:
    res = run_cores(inputs)
    out = np.concatenate(
        [res.results[ci]["out"].astype(np.float32).reshape(BL, C, H, W)
         for ci in range(N_CORES)],
        axis=0)
    return out
